# revision 5
# baseline (speedup 1.0000x reference)
"""GCMC layer Bass kernel v4 — dma_gather-based, 8 TRN2 NeuronCores.

Math per direction/rating:
  out_dst[m, r, :] = ci_dst[m] * (sum_{e: dst=m} y_src_r[src_e]) + fc_b
  where y_side_r = (cj_side * side_feat) @ Wfc_r,  Wfc_r = (sum_b att[r,b] basis[b]) @ fc_w

Strategy (v4):
  - Launch 1 (row-sharded): per-(side, rating) transformed node tables in
    PAIR layout: pairtab[p] = [y[2p] | y[2p+1]]  ([N/2, 128] bf16), so gather
    indices fit int16 at dma_gather's 256-byte row minimum.
  - Launch 2 (dest-sharded) is dma_gather descriptor-rate bound on HW
    (~4.7 ns/descriptor with >=2 SWDGE queues; bytes per descriptor are
    irrelevant).  v4 therefore minimizes descriptors:
      * cross-tile chunking: the per-(d, r) dest-sorted edge stream is cut
        into 128-edge chunks that run across dest-tile boundaries; chunk
        padding only at every ALIGN-tile group boundary, to the max-over-
        cores group count, so all cores share one chunk layout (SPMD) at
        ~3% padding instead of the ~15% of per-(d,r,tile) chunk ceils.
      * gathers are spread over NQ=2 SWDGE queues (single-queue transfer
        is the 7.9 ns/desc bottleneck; 2 queues reach the ~4.7 ns/desc
        descriptor-generation floor).
    Per dest tile the program processes the shared chunk span [jmin, jmax)
    (min/max over cores); per-core dstloc columns carry 255 sentinels for
    edges outside the tile so their one-hot columns are zero.
  - P build uses the [f, c] layout with a materialized iota2 so every DVE
    operand's last dim is packed 2-byte (2x_1p mode); fully hidden behind
    the gathers.
"""

import json
import os
import time

import numpy as np
import ml_dtypes

_VERBOSE = os.environ.get("KERNEL_VERBOSE", "0") == "1"


def _tlog(msg, t0=[None]):
    if _VERBOSE:
        now = time.time()
        dt = 0.0 if t0[0] is None else now - t0[0]
        t0[0] = now
        print(f"[kernel +{dt:6.2f}s] {msg}", flush=True)


import concourse.bass as bass
import concourse.mybir as mybir
import concourse.tile as tile
from concourse.bass_utils import run_bass_kernel_spmd
from concourse.library_config import all_libraries, standard
import bass_rust as _bass_rust

BF16 = ml_dtypes.bfloat16


# ----------------------------------------------------------------------
# Walrus workaround: split multi-wait sync into standalone EventSemaphores
# (the staged walrus rejects >1 sync wait per instruction).
# ----------------------------------------------------------------------

def _split_multiwaits(bir: bytes) -> bytes:
    j = json.loads(bir)
    for fn in j["functions"]:
        for blk in fn["blocks"]:
            out = []
            k = 0
            for ins in blk["instructions"]:
                si = ins.get("sync_info") or {}
                waits = si.get("on_wait") or []
                if len(waits) > 1:
                    for w in waits[:-1]:
                        out.append({
                            "debug": ins.get("debug"),
                            "engine": ins["engine"],
                            "ins": [], "outs": [],
                            "name": f"{ins['name']}-ws{k}",
                            "opcode": "EventSemaphore",
                            "sync_info": {"on_update": [], "on_wait": [w]},
                        })
                        k += 1
                    si["on_wait"] = [waits[-1]]
                out.append(ins)
            blk["instructions"] = out
    return json.dumps(j).encode()


_orig_to_json_bytes = bass.Bass.to_json_bytes


def _patched_to_json_bytes(self):
    return _split_multiwaits(_orig_to_json_bytes(self))


bass.Bass.to_json_bytes = _patched_to_json_bytes


def _finalize_libraries(nc):
    """Bacc-style library-load insertion + extended-inst ISA codegen for raw
    Bass (needed for dma_gather / InstDMAGatherAnt)."""
    inst_type_to_lib_mask = {}
    for lib in all_libraries:
        for inst_type in lib.instructions:
            inst_type_to_lib_mask[inst_type] = inst_type_to_lib_mask.get(
                inst_type, 0) | (1 << lib.index)
    _bass_rust.insert_library_loads(
        nc, inst_type_to_lib_mask, len(all_libraries), standard.index)
    mybir.codegen_inst_isa_subclasses(nc)


# ----- problem constants -----
N = 50000          # nodes per side
F = 128            # feature dim
R = 5              # ratings
E = 400000         # edges per rating per direction
OUT = 64           # output dim
NB = 2             # basis count
NCORES = 8

WCH = 32           # chunks per dma_gather window
ALIGN = 4          # dest-tiles per shared chunk-group
NQ = 2             # SWDGE queues
PSTYLE = "swap"    # P one-hot build: "swap" (DVE 2x, needs iota2 SBUF) or
                   # "base" (1x, no iota2; still hidden behind the gathers)

f32 = mybir.dt.float32
bf16 = mybir.dt.bfloat16
i16 = mybir.dt.int16

NPC = N // NCORES                  # dests per core (6250)
NT = (NPC + 127) // 128            # dest tiles per core (49)
NPAD = NT * 128                    # padded dests per core (6272)
NPAIR_C = NPC // 2                 # real pair rows per core (3125)
NPAIR_CP = NT * 64                 # padded pair rows per core slice (3136)
NPAIR = NPAIR_C * NCORES           # total real pair rows (25000)
NPAIRPAD = ((NPAIR + 63) // 64) * 64
PAD_DL = 255                       # dstloc sentinel -> all-zero one-hot col


# ======================================================================
# Host-side edge preprocessing (v4: shared-group cross-tile chunking)
# ======================================================================

def prep_edges(edge_drug, edge_dis):
    """Returns NCH[d,r] (shared chunk counts), spans[d,r,t] = (jmin, jmax+1)
    shared per-tile chunk spans, data[d][r][c] = (src, dl) padded per-core
    edge streams (pad entries: dl=-1)."""
    NG = (NT + ALIGN - 1) // ALIGN
    data = [[None] * R for _ in range(2)]
    NCH = np.zeros((2, R), np.int64)
    spans = np.zeros((2, R, NT, 2), np.int64)
    spans[:, :, :, 0] = 10 ** 9

    for d in range(2):
        src_all, dst_all = ((edge_drug, edge_dis) if d == 0
                            else (edge_dis, edge_drug))
        for r in range(R):
            order = np.argsort(dst_all[r], kind="stable")
            dst_s = dst_all[r][order].astype(np.int64)
            src_s = src_all[r][order].astype(np.int64)
            bounds = np.searchsorted(dst_s, np.arange(NCORES + 1) * NPC)
            raw = []
            gcnt = np.zeros((NCORES, NG), np.int64)
            for c in range(NCORES):
                lo, hi = bounds[c], bounds[c + 1]
                src_c = src_s[lo:hi]
                dl_c = dst_s[lo:hi] - c * NPC
                gid = (dl_c >> 7) // ALIGN
                gcnt[c] = np.bincount(gid, minlength=NG)
                raw.append((src_c, dl_c))
            G = (gcnt.max(axis=0) + 127) // 128
            goff = np.zeros(NG + 1, np.int64)
            np.cumsum(G, out=goff[1:])
            NCH[d, r] = goff[NG]

            percore = []
            for c in range(NCORES):
                src_c, dl_c = raw[c]
                gid = (dl_c >> 7) // ALIGN
                gb = np.searchsorted(gid, np.arange(NG + 1))
                ps, pd = [], []
                for g in range(NG):
                    seg_s = src_c[gb[g]:gb[g + 1]]
                    seg_d = dl_c[gb[g]:gb[g + 1]]
                    pad = int(G[g]) * 128 - len(seg_s)
                    ps.append(seg_s)
                    pd.append(seg_d)
                    if pad:
                        ps.append(np.zeros(pad, np.int64))
                        pd.append(np.full(pad, -1, np.int64))
                    tid = seg_d >> 7
                    for t in range(g * ALIGN, min((g + 1) * ALIGN, NT)):
                        i0 = np.searchsorted(tid, t, side="left")
                        i1 = np.searchsorted(tid, t, side="right")
                        if i1 <= i0:
                            continue
                        j0 = goff[g] + i0 // 128
                        j1 = goff[g] + (i1 - 1) // 128 + 1
                        spans[d, r, t, 0] = min(spans[d, r, t, 0], j0)
                        spans[d, r, t, 1] = max(spans[d, r, t, 1], j1)
                percore.append((np.concatenate(ps), np.concatenate(pd)))
            data[d][r] = percore

    bad = spans[:, :, :, 0] >= spans[:, :, :, 1]
    spans[:, :, :, 0][bad] = 0
    spans[:, :, :, 1][bad] = 1
    return NCH, spans, data


def build_core_arrays(NCH, spans, data, c):
    """idx16 [128, icols] i16 and dstloc [128, dcols] bf16 for core c."""
    ICW = WCH * 128 // 16
    idx_parts = []
    for d in range(2):
        for r in range(R):
            src, dl = data[d][r][c]
            nch = int(NCH[d, r])
            nw = (nch + WCH - 1) // WCH
            buf = np.zeros(nw * WCH * 128, np.int64)
            buf[:len(src)] = src >> 1
            lin = buf.reshape(nw, WCH * 128)
            wrap = lin.reshape(nw, ICW, 16).transpose(0, 2, 1)
            blk = np.tile(wrap, (1, 8, 1))
            idx_parts.append(blk.transpose(1, 0, 2).reshape(128, nw * ICW))
    idx16 = np.ascontiguousarray(
        np.concatenate(idx_parts, axis=1).astype(np.int16))

    dl_parts = []
    for d in range(2):
        for t in range(NT):
            for r in range(R):
                src, dl = data[d][r][c]
                j0, j1 = int(spans[d, r, t, 0]), int(spans[d, r, t, 1])
                for j in range(j0, j1):
                    ed = dl[j * 128:(j + 1) * 128]
                    es = src[j * 128:(j + 1) * 128]
                    in_tile = (ed >= 0) & ((ed >> 7) == t)
                    be = np.full(128, PAD_DL, np.int64)
                    bo = np.full(128, PAD_DL, np.int64)
                    par = (es & 1).astype(bool)
                    sel_e = in_tile & ~par
                    sel_o = in_tile & par
                    be[sel_e] = ed[sel_e] & 127
                    bo[sel_o] = ed[sel_o] & 127
                    dl_parts.append(np.stack([be, bo], axis=0).T)
    dstloc = np.ascontiguousarray(
        np.concatenate(dl_parts, axis=1).astype(BF16))
    return idx16, dstloc


# ======================================================================
# Launch 1: pair tables  y_{side,r}  [NPAIRPAD, 128] bf16
# ======================================================================

def build_prep_nc():
    nc = bass.Bass()
    feat_in = nc.dram_tensor("feat_slice", (2, NPAD, F), f32, kind="ExternalInput")
    cj_in = nc.dram_tensor("cj_slice", (2, 128, NT), f32, kind="ExternalInput")
    att_in = nc.dram_tensor("att", (R, NB), f32, kind="ExternalInput")
    basis_in = nc.dram_tensor("basis", (NB, F, F), f32, kind="ExternalInput")
    fcw_in = nc.dram_tensor("fc_w", (F, OUT), f32, kind="ExternalInput")
    y_out = nc.dram_tensor("y_slice", (2, R, NPAIR_CP, 128), bf16,
                           kind="ExternalOutput")

    ident_c = nc.inline_tensor(np.eye(128, dtype=np.float32), "ident_c")
    ones_c = nc.inline_tensor(np.ones((1, 128), dtype=np.float32), "ones_c")
    mult = mybir.AluOpType.mult
    add = mybir.AluOpType.add

    with tile.TileContext(nc) as tc:
        with (
            tc.tile_pool(name="cp", bufs=1) as cp,
            tc.tile_pool(name="sb", bufs=4) as sb,
            tc.tile_pool(name="xp", bufs=6) as xp,
            tc.tile_pool(name="ps", bufs=2, space="PSUM") as ps,
            tc.tile_pool(name="ps2", bufs=2, space="PSUM") as ps2,
        ):
            ident_t = cp.tile([128, 128], f32, tag="ident")
            nc.sync.dma_start(out=ident_t[:], in_=ident_c[:, :])
            ones_f32 = cp.tile([1, 128], f32, tag="ones32")
            nc.sync.dma_start(out=ones_f32[:], in_=ones_c[:, :])
            fcw_t = cp.tile([128, OUT], f32, tag="fcw")
            nc.sync.dma_start(out=fcw_t[:], in_=fcw_in[:, :])
            att_row = cp.tile([1, R * NB], f32, tag="attrow")
            nc.sync.dma_start(out=att_row[:],
                              in_=att_in[:, :].rearrange("r b -> () (r b)"))
            cj_sb = cp.tile([128, 2 * NT], f32, tag="cj")
            nc.sync.dma_start(
                out=cj_sb[:].rearrange("p (s t) -> p s t", s=2),
                in_=cj_in[:, :, :].rearrange("s p t -> p s t"))

            attb_ps = ps2.tile([128, R * NB], f32, tag="o2")
            nc.tensor.matmul(out=attb_ps[:], lhsT=ones_f32[:], rhs=att_row[:],
                             start=True, stop=True)
            att_b = cp.tile([128, R * NB], f32, tag="attb")
            nc.vector.tensor_copy(out=att_b[:], in_=attb_ps[:])

            bT = []
            for b in range(NB):
                bt_in = sb.tile([128, 128], f32, tag="bload")
                nc.sync.dma_start(out=bt_in[:], in_=basis_in[b, :, :])
                bt_ps = ps.tile([128, 128], f32, tag="tp")
                nc.tensor.transpose(out=bt_ps[:], in_=bt_in[:], identity=ident_t[:])
                bt_sb = cp.tile([128, 128], f32, tag=f"bT{b}")
                nc.vector.tensor_copy(out=bt_sb[:], in_=bt_ps[:])
                bT.append(bt_sb)

            wfc = cp.tile([128, R * OUT], bf16, tag="wfc")
            for r in range(R):
                wrt = sb.tile([128, 128], f32, tag="wrt")
                tmp = sb.tile([128, 128], f32, tag="wtmp")
                nc.vector.tensor_tensor(
                    out=tmp[:], in0=bT[1][:],
                    in1=att_b[:, 2 * r + 1:2 * r + 2].to_broadcast([128, 128]),
                    op=mult)
                nc.vector.tensor_tensor(
                    out=wrt[:], in0=bT[0][:],
                    in1=att_b[:, 2 * r:2 * r + 1].to_broadcast([128, 128]),
                    op=mult)
                nc.vector.tensor_tensor(out=wrt[:], in0=wrt[:], in1=tmp[:], op=add)
                wfc_ps = ps2.tile([128, OUT], f32, tag="o2")
                nc.tensor.matmul(out=wfc_ps[:], lhsT=wrt[:], rhs=fcw_t[:],
                                 start=True, stop=True)
                nc.scalar.copy(out=wfc[:, r * OUT:(r + 1) * OUT], in_=wfc_ps[:])

            for s in range(2):
                for t in range(NT):
                    rows = slice(t * 128, (t + 1) * 128)
                    ft = xp.tile([128, F], f32, tag="ft")
                    nc.sync.dma_start(out=ft[:], in_=feat_in[s, rows, :])
                    xt = xp.tile([128, F], f32, tag="xt")
                    nc.vector.tensor_tensor(
                        out=xt[:], in0=ft[:],
                        in1=cj_sb[:, s * NT + t:s * NT + t + 1].to_broadcast(
                            [128, F]),
                        op=mult)
                    xT_ps = ps.tile([128, 128], f32, tag="tp")
                    nc.tensor.transpose(out=xT_ps[:], in_=xt[:],
                                        identity=ident_t[:])
                    xT = xp.tile([128, 128], bf16, tag="xT")
                    nc.scalar.copy(out=xT[:], in_=xT_ps[:])
                    y_ps = ps.tile([128, R * OUT], f32, tag="y")
                    nc.tensor.matmul(out=y_ps[:], lhsT=xT[:], rhs=wfc[:],
                                     start=True, stop=True)
                    y_sb = xp.tile([128, R * OUT], bf16, tag="ysb")
                    nc.vector.tensor_copy(out=y_sb[:], in_=y_ps[:])
                    nc.sync.dma_start(
                        out=y_out[s, :, t * 64:(t + 1) * 64, :].rearrange(
                            "r q (h o) -> (q h) r o", h=2),
                        in_=y_sb[:].rearrange("p (r o) -> p r o", r=R))
    return nc


# ======================================================================
# Launch 2: main kernel (v4)
# ======================================================================

def build_main_nc(NCH, spans, icols, dcols, reps=1):
    nwin = [[int((NCH[d, r] + WCH - 1) // WCH) for r in range(R)]
            for d in range(2)]
    nc2t_all = [[2 * int(sum(spans[d, r, t, 1] - spans[d, r, t, 0]
                             for r in range(R)))
                 for t in range(NT)] for d in range(2)]
    nc2t_max = max(max(row) for row in nc2t_all)

    nc = bass.Bass(num_swdge_queues=NQ)
    ytab = [[nc.dram_tensor(f"y{s}{r}", (NPAIRPAD, 128), bf16,
                            kind="ExternalInput")
             for r in range(R)] for s in range(2)]
    idx_in = nc.dram_tensor("idx16", (128, icols), i16, kind="ExternalInput")
    dl_in = nc.dram_tensor("dstloc", (128, dcols), bf16, kind="ExternalInput")
    ci_in = nc.dram_tensor("ci_pad", (2, 128, NT), f32, kind="ExternalInput")
    fcb_in = nc.dram_tensor("fc_b", (OUT,), f32, kind="ExternalInput")
    out = nc.dram_tensor("out_part", (2, NPAD, R, OUT), f32,
                         kind="ExternalOutput")

    iota_np = np.broadcast_to(np.arange(128, dtype=np.float32), (128, 128))
    iota_c = nc.inline_tensor(np.ascontiguousarray(iota_np), "iota_c")
    ones_c = nc.inline_tensor(np.ones((1, 128), dtype=np.float32), "ones_c")

    eq = mybir.AluOpType.is_equal
    add = mybir.AluOpType.add
    NI = WCH * 128
    ICW = NI // 16

    with tile.TileContext(nc) as tc:
        with (
            tc.tile_pool(name="cp", bufs=1) as cp,
            tc.tile_pool(name="ixp", bufs=4) as ixp,
            tc.tile_pool(name="mp0", bufs=2) as mp0,
            tc.tile_pool(name="mp1", bufs=2) as mp1,
            tc.tile_pool(name="mp2", bufs=2) as mp2,
            tc.tile_pool(name="mp3", bufs=2) as mp3,
            tc.tile_pool(name="mp4", bufs=2) as mp4,
            tc.tile_pool(name="pp", bufs=2) as pp,
            tc.tile_pool(name="obp", bufs=4) as obp,
            tc.tile_pool(name="ps2", bufs=4, space="PSUM") as ps2,
        ):
            mps = [mp0, mp1, mp2, mp3, mp4]
            iota_f = cp.tile([128, 128], f32, tag="iotaf")
            nc.sync.dma_start(out=iota_f[:], in_=iota_c[:, :])
            iota_t = cp.tile([128, 128], bf16, tag="iota")
            nc.vector.tensor_copy(out=iota_t[:], in_=iota_f[:])
            iota2 = None
            if PSTYLE == "swap":
                iota2 = cp.tile([128, 128, nc2t_max], bf16, tag="iota2")
                nc.vector.tensor_copy(
                    out=iota2[:],
                    in_=iota_t[:, :, None].to_broadcast([128, 128, nc2t_max]))
            ones_f32 = cp.tile([1, 128], f32, tag="ones32")
            nc.sync.dma_start(out=ones_f32[:], in_=ones_c[:, :])
            fcb_row = cp.tile([1, OUT], f32, tag="fcbrow")
            nc.sync.dma_start(out=fcb_row[:], in_=fcb_in[None, :])
            ci_sb = cp.tile([128, 2 * NT], f32, tag="ci")
            nc.sync.dma_start(
                out=ci_sb[:].rearrange("p (s t) -> p s t", s=2),
                in_=ci_in[:, :, :].rearrange("s p t -> p s t"))
            dl_t = cp.tile([128, dcols], bf16, tag="dl")
            nc.sync.dma_start(out=dl_t[:], in_=dl_in[:, :])

            biasb_ps = ps2.tile([128, OUT], f32, tag="o2s")
            nc.tensor.matmul(out=biasb_ps[:], lhsT=ones_f32[:], rhs=fcb_row[:],
                             start=True, stop=True)
            bias5 = cp.tile([128, R * OUT], f32, tag="bias5")
            for r in range(R):
                nc.vector.tensor_copy(out=bias5[:, r * OUT:(r + 1) * OUT],
                                      in_=biasb_ps[:])

            ni_regs = {NI: nc.gpsimd.to_reg(NI)}
            for d in range(2):
                for r in range(R):
                    nch = int(NCH[d, r])
                    w = nwin[d][r] - 1
                    ni = (min(WCH, nch - w * WCH)) * 128
                    if ni not in ni_regs:
                        ni_regs[ni] = nc.gpsimd.to_reg(ni)

            import contextlib
            loop_cm = tc.For_i(0, reps) if reps > 1 else contextlib.nullcontext()
            with loop_cm:
                iblk = 0
                dcol = 0
                qn = 0
                for d in range(2):
                    mt = [[None] * nwin[d][r] for r in range(R)]
                    wmax = max(nwin[d][r] for r in range(R))
                    boff = [iblk + sum(nwin[d][rr] for rr in range(r))
                            for r in range(R)]
                    for w in range(wmax):
                        for r in range(R):
                            if w >= nwin[d][r]:
                                continue
                            nch = int(NCH[d, r])
                            wch = (min(WCH, nch - w * WCH)
                                   if w == nwin[d][r] - 1 else WCH)
                            ni = wch * 128
                            ix = ixp.tile([128, ICW], i16, tag="ix")
                            nc.sync.dma_start(
                                out=ix[:, :ni // 16],
                                in_=idx_in[:, (boff[r] + w) * ICW:
                                           (boff[r] + w) * ICW + ni // 16])
                            m = mps[r].tile([128, WCH * 128], bf16, tag="m")
                            nc.gpsimd.dma_gather(
                                m[:, :wch * 128].rearrange(
                                    "p (c f) -> p c f", c=wch),
                                ytab[d][r][:, :],
                                ix[:, :ni // 16],
                                ni, ni_regs[ni], 128,
                                single_packet=False,
                                queue_num=qn % NQ)
                            qn += 1
                            mt[r][w] = m
                    iblk += sum(nwin[d][r] for r in range(R))

                    p_tiles = {}
                    dcols_t = [dcol]
                    for t in range(NT):
                        dcols_t.append(dcols_t[-1] + nc2t_all[d][t])

                    def build_p(t):
                        nc2t = nc2t_all[d][t]
                        if PSTYLE == "swap":
                            p_t = pp.tile([128, 128, nc2t_max], bf16, tag="p")
                            nc.vector.tensor_tensor(
                                out=p_t[:, :, :nc2t],
                                in0=dl_t[:, dcols_t[t]:dcols_t[t] + nc2t]
                                [:, None, :].to_broadcast([128, 128, nc2t]),
                                in1=iota2[:, :, :nc2t],
                                op=eq)
                        else:
                            p_t = pp.tile([128, nc2t_max, 128], bf16, tag="p")
                            nc.vector.tensor_tensor(
                                out=p_t[:, :nc2t, :],
                                in0=dl_t[:, dcols_t[t]:dcols_t[t] + nc2t]
                                [:, :, None].to_broadcast([128, nc2t, 128]),
                                in1=iota_t[:, None, :].to_broadcast(
                                    [128, nc2t, 128]),
                                op=eq)
                        p_tiles[t] = p_t

                    def p_col(p_t, col):
                        if PSTYLE == "swap":
                            return p_t[:, :, col]
                        return p_t[:, col, :]

                    build_p(0)
                    for t in range(NT):
                        if t + 1 < NT:
                            build_p(t + 1)
                        p_t = p_tiles.pop(t)
                        o2 = ps2.tile([128, R * OUT], f32, tag="o2")
                        pc = 0
                        for r in range(R):
                            j0 = int(spans[d, r, t, 0])
                            j1 = int(spans[d, r, t, 1])
                            k = j1 - j0
                            for jj in range(k):
                                j = j0 + jj
                                w, q = divmod(j, WCH)
                                for par in range(2):
                                    nc.tensor.matmul(
                                        out=o2[:, r * OUT:(r + 1) * OUT],
                                        lhsT=p_col(p_t, 2 * (pc + jj) + par),
                                        rhs=mt[r][w][:, q * 128 + par * 64:
                                                     q * 128 + par * 64 + 64],
                                        start=(jj == 0 and par == 0),
                                        stop=(jj == k - 1 and par == 1))
                            pc += k
                        dcol += 2 * pc
                        ob = obp.tile([128, R * OUT], f32, tag="ob")
                        nc.scalar.activation(
                            out=ob[:], in_=o2[:],
                            func=mybir.ActivationFunctionType.Copy,
                            scale=ci_sb[:, d * NT + t:d * NT + t + 1])
                        nc.vector.tensor_tensor(out=ob[:], in0=ob[:],
                                                in1=bias5[:], op=add)
                        nc.scalar.dma_start(
                            out=out[d, t * 128:(t + 1) * 128, :, :].rearrange(
                                "p r o -> p (r o)"),
                            in_=ob[:])
    _finalize_libraries(nc)
    return nc


# ======================================================================
# kernel entry
# ======================================================================

_cache: dict = {}


def kernel(drug_feat, dis_feat, cj_drug, ci_drug, cj_dis, ci_dis,
           att, basis, fc_w, fc_b, edge_drug, edge_dis):
    drug_feat = np.asarray(drug_feat, np.float32)
    dis_feat = np.asarray(dis_feat, np.float32)
    cj_drug = np.asarray(cj_drug, np.float32)
    ci_drug = np.asarray(ci_drug, np.float32)
    cj_dis = np.asarray(cj_dis, np.float32)
    ci_dis = np.asarray(ci_dis, np.float32)
    att = np.asarray(att, np.float32)
    basis = np.asarray(basis, np.float32)
    fc_w = np.asarray(fc_w, np.float32)
    fc_b = np.asarray(fc_b, np.float32)
    edge_drug = np.asarray(edge_drug, np.int32)
    edge_dis = np.asarray(edge_dis, np.int32)

    _tlog("start")
    NCH, spans, data = prep_edges(edge_drug, edge_dis)
    arrays = [build_core_arrays(NCH, spans, data, c) for c in range(NCORES)]
    icols = arrays[0][0].shape[1]
    dcols = arrays[0][1].shape[1]
    _tlog(f"host edge prep done (chunks={int(NCH.sum())}, icols={icols})")

    # ---- launch 1 ----
    if "prep" not in _cache:
        _cache["prep"] = build_prep_nc()
    nc1 = _cache["prep"]
    in_maps1 = []
    for c in range(NCORES):
        rows = slice(c * NPC, (c + 1) * NPC)
        feat_slice = np.zeros((2, NPAD, F), np.float32)
        feat_slice[0, :NPC] = drug_feat[rows]
        feat_slice[1, :NPC] = dis_feat[rows]
        cj_slice = np.zeros((2, NPAD), np.float32)
        cj_slice[0, :NPC] = cj_drug[rows]
        cj_slice[1, :NPC] = cj_dis[rows]
        cj_slice = np.ascontiguousarray(
            cj_slice.reshape(2, NT, 128).transpose(0, 2, 1))
        in_maps1.append({"feat_slice": feat_slice, "cj_slice": cj_slice,
                         "att": att, "basis": basis, "fc_w": fc_w})
    _tlog("launch1 inputs built")
    res1 = run_bass_kernel_spmd(nc1, in_maps1, core_ids=list(range(NCORES)))
    _tlog("launch1 done")

    ytabs = np.zeros((2, R, NPAIRPAD, 128), BF16)
    for c in range(NCORES):
        ytabs[:, :, c * NPAIR_C:(c + 1) * NPAIR_C, :] = \
            res1.results[c]["y_slice"][:, :, :NPAIR_C]
    _tlog("tables assembled")

    # ---- launch 2 ----
    key = ("main", NCH.tobytes(), spans.tobytes(), icols, dcols)
    if key not in _cache:
        _cache[key] = build_main_nc(NCH, spans, icols, dcols)
        _tlog("launch2 program built")
    nc2 = _cache[key]

    in_maps2 = []
    for c in range(NCORES):
        rows = slice(c * NPC, (c + 1) * NPC)
        ci_pad = np.zeros((2, NPAD), np.float32)
        ci_pad[0, :NPC] = ci_dis[rows]    # dir 0 dest = dis
        ci_pad[1, :NPC] = ci_drug[rows]   # dir 1 dest = drug
        ci_pad = np.ascontiguousarray(
            ci_pad.reshape(2, NT, 128).transpose(0, 2, 1))
        im = {"idx16": arrays[c][0], "dstloc": arrays[c][1],
              "ci_pad": ci_pad, "fc_b": fc_b}
        for s in range(2):
            for r in range(R):
                im[f"y{s}{r}"] = ytabs[s, r]
        in_maps2.append(im)
    _tlog("launch2 inputs built")
    res2 = run_bass_kernel_spmd(nc2, in_maps2, core_ids=list(range(NCORES)))
    _tlog("launch2 done")

    out_dis = np.concatenate(
        [r["out_part"][0, :NPC] for r in res2.results], axis=0)
    out_drug = np.concatenate(
        [r["out_part"][1, :NPC] for r in res2.results], axis=0)
    _tlog("assembled")
    return out_drug.astype(np.float32), out_dis.astype(np.float32)


# revision 12
# speedup vs baseline: 1.0029x; 1.0029x over previous
"""GCMC layer Bass kernel v4 — dma_gather-based, 8 TRN2 NeuronCores.

Math per direction/rating:
  out_dst[m, r, :] = ci_dst[m] * (sum_{e: dst=m} y_src_r[src_e]) + fc_b
  where y_side_r = (cj_side * side_feat) @ Wfc_r,  Wfc_r = (sum_b att[r,b] basis[b]) @ fc_w

Strategy (v4):
  - Launch 1 (row-sharded): per-(side, rating) transformed node tables in
    PAIR layout: pairtab[p] = [y[2p] | y[2p+1]]  ([N/2, 128] bf16), so gather
    indices fit int16 at dma_gather's 256-byte row minimum.
  - Launch 2 (dest-sharded) is dma_gather descriptor-rate bound on HW
    (~4.7 ns/descriptor with >=2 SWDGE queues; bytes per descriptor are
    irrelevant).  v4 therefore minimizes descriptors:
      * cross-tile chunking: the per-(d, r) dest-sorted edge stream is cut
        into 128-edge chunks that run across dest-tile boundaries; chunk
        padding only at every ALIGN-tile group boundary, to the max-over-
        cores group count, so all cores share one chunk layout (SPMD) at
        ~3% padding instead of the ~15% of per-(d,r,tile) chunk ceils.
      * gathers are spread over NQ=2 SWDGE queues (single-queue transfer
        is the 7.9 ns/desc bottleneck; 2 queues reach the ~4.7 ns/desc
        descriptor-generation floor).
    Per dest tile the program processes the shared chunk span [jmin, jmax)
    (min/max over cores); per-core dstloc columns carry 255 sentinels for
    edges outside the tile so their one-hot columns are zero.
  - P build uses the [f, c] layout with a materialized iota2 so every DVE
    operand's last dim is packed 2-byte (2x_1p mode); fully hidden behind
    the gathers.
"""

import json
import os
import time

import numpy as np
import ml_dtypes

_VERBOSE = os.environ.get("KERNEL_VERBOSE", "0") == "1"


def _tlog(msg, t0=[None]):
    if _VERBOSE:
        now = time.time()
        dt = 0.0 if t0[0] is None else now - t0[0]
        t0[0] = now
        print(f"[kernel +{dt:6.2f}s] {msg}", flush=True)


import concourse.bass as bass
import concourse.mybir as mybir
import concourse.tile as tile
from concourse.bass_utils import run_bass_kernel_spmd
from concourse.library_config import all_libraries, standard
import bass_rust as _bass_rust

BF16 = ml_dtypes.bfloat16


# ----------------------------------------------------------------------
# Walrus workaround: split multi-wait sync into standalone EventSemaphores
# (the staged walrus rejects >1 sync wait per instruction).
# ----------------------------------------------------------------------

def _split_multiwaits(bir: bytes) -> bytes:
    j = json.loads(bir)
    for fn in j["functions"]:
        for blk in fn["blocks"]:
            out = []
            k = 0
            for ins in blk["instructions"]:
                si = ins.get("sync_info") or {}
                waits = si.get("on_wait") or []
                if len(waits) > 1:
                    for w in waits[:-1]:
                        out.append({
                            "debug": ins.get("debug"),
                            "engine": ins["engine"],
                            "ins": [], "outs": [],
                            "name": f"{ins['name']}-ws{k}",
                            "opcode": "EventSemaphore",
                            "sync_info": {"on_update": [], "on_wait": [w]},
                        })
                        k += 1
                    si["on_wait"] = [waits[-1]]
                out.append(ins)
            blk["instructions"] = out
    return json.dumps(j).encode()


_orig_to_json_bytes = bass.Bass.to_json_bytes


def _patched_to_json_bytes(self):
    return _split_multiwaits(_orig_to_json_bytes(self))


bass.Bass.to_json_bytes = _patched_to_json_bytes


def _finalize_libraries(nc):
    """Bacc-style library-load insertion + extended-inst ISA codegen for raw
    Bass (needed for dma_gather / InstDMAGatherAnt)."""
    inst_type_to_lib_mask = {}
    for lib in all_libraries:
        for inst_type in lib.instructions:
            inst_type_to_lib_mask[inst_type] = inst_type_to_lib_mask.get(
                inst_type, 0) | (1 << lib.index)
    _bass_rust.insert_library_loads(
        nc, inst_type_to_lib_mask, len(all_libraries), standard.index)
    mybir.codegen_inst_isa_subclasses(nc)


# ----- problem constants -----
N = 50000          # nodes per side
F = 128            # feature dim
R = 5              # ratings
E = 400000         # edges per rating per direction
OUT = 64           # output dim
NB = 2             # basis count
NCORES = 8

WCH = 32           # chunks per dma_gather window
ALIGN = 4          # dest-tiles per shared chunk-group
NQ = 2             # SWDGE queues
PSTYLE = "swap"    # P one-hot build: "swap" (DVE 2x, needs iota2 SBUF) or
                   # "base" (1x, no iota2; still hidden behind the gathers)

f32 = mybir.dt.float32
bf16 = mybir.dt.bfloat16
i16 = mybir.dt.int16

NPC = N // NCORES                  # dests per core (6250)
NT = (NPC + 127) // 128            # dest tiles per core (49)
NPAD = NT * 128                    # padded dests per core (6272)
NPAIR_C = NPC // 2                 # real pair rows per core (3125)
NPAIR_CP = NT * 64                 # padded pair rows per core slice (3136)
NPAIR = NPAIR_C * NCORES           # total real pair rows (25000)
NPAIRPAD = ((NPAIR + 63) // 64) * 64
PAD_DL = 255                       # dstloc sentinel -> all-zero one-hot col


# ======================================================================
# Host-side edge preprocessing (v4: shared-group cross-tile chunking)
# ======================================================================

def prep_edges(edge_drug, edge_dis):
    """Returns NCH[d,r] (shared chunk counts), spans[d,r,t] = (jmin, jmax+1)
    shared per-tile chunk spans, data[d][r][c] = (src, dl) padded per-core
    edge streams (pad entries: dl=-1)."""
    NG = (NT + ALIGN - 1) // ALIGN
    data = [[None] * R for _ in range(2)]
    NCH = np.zeros((2, R), np.int64)
    spans = np.zeros((2, R, NT, 2), np.int64)
    spans[:, :, :, 0] = 10 ** 9

    for d in range(2):
        src_all, dst_all = ((edge_drug, edge_dis) if d == 0
                            else (edge_dis, edge_drug))
        for r in range(R):
            order = np.argsort(dst_all[r], kind="stable")
            dst_s = dst_all[r][order].astype(np.int64)
            src_s = src_all[r][order].astype(np.int64)
            bounds = np.searchsorted(dst_s, np.arange(NCORES + 1) * NPC)
            raw = []
            gcnt = np.zeros((NCORES, NG), np.int64)
            for c in range(NCORES):
                lo, hi = bounds[c], bounds[c + 1]
                src_c = src_s[lo:hi]
                dl_c = dst_s[lo:hi] - c * NPC
                gid = (dl_c >> 7) // ALIGN
                gcnt[c] = np.bincount(gid, minlength=NG)
                raw.append((src_c, dl_c))
            G = (gcnt.max(axis=0) + 127) // 128
            goff = np.zeros(NG + 1, np.int64)
            np.cumsum(G, out=goff[1:])
            NCH[d, r] = goff[NG]

            percore = []
            for c in range(NCORES):
                src_c, dl_c = raw[c]
                gid = (dl_c >> 7) // ALIGN
                gb = np.searchsorted(gid, np.arange(NG + 1))
                ps, pd = [], []
                for g in range(NG):
                    seg_s = src_c[gb[g]:gb[g + 1]]
                    seg_d = dl_c[gb[g]:gb[g + 1]]
                    pad = int(G[g]) * 128 - len(seg_s)
                    ps.append(seg_s)
                    pd.append(seg_d)
                    if pad:
                        ps.append(np.zeros(pad, np.int64))
                        pd.append(np.full(pad, -1, np.int64))
                    tid = seg_d >> 7
                    for t in range(g * ALIGN, min((g + 1) * ALIGN, NT)):
                        i0 = np.searchsorted(tid, t, side="left")
                        i1 = np.searchsorted(tid, t, side="right")
                        if i1 <= i0:
                            continue
                        j0 = goff[g] + i0 // 128
                        j1 = goff[g] + (i1 - 1) // 128 + 1
                        spans[d, r, t, 0] = min(spans[d, r, t, 0], j0)
                        spans[d, r, t, 1] = max(spans[d, r, t, 1], j1)
                percore.append((np.concatenate(ps), np.concatenate(pd)))
            data[d][r] = percore

    bad = spans[:, :, :, 0] >= spans[:, :, :, 1]
    spans[:, :, :, 0][bad] = 0
    spans[:, :, :, 1][bad] = 1
    return NCH, spans, data


def build_core_arrays(NCH, spans, data, c):
    """idx16 [128, icols] i16 and dstloc [128, dcols] bf16 for core c."""
    ICW = WCH * 128 // 16
    idx_parts = []
    for d in range(2):
        for r in range(R):
            src, dl = data[d][r][c]
            nch = int(NCH[d, r])
            nw = (nch + WCH - 1) // WCH
            buf = np.zeros(nw * WCH * 128, np.int64)
            buf[:len(src)] = src >> 1
            lin = buf.reshape(nw, WCH * 128)
            wrap = lin.reshape(nw, ICW, 16).transpose(0, 2, 1)
            blk = np.tile(wrap, (1, 8, 1))
            idx_parts.append(blk.transpose(1, 0, 2).reshape(128, nw * ICW))
    idx16 = np.ascontiguousarray(
        np.concatenate(idx_parts, axis=1).astype(np.int16))

    dl_parts = []
    for d in range(2):
        for t in range(NT):
            for r in range(R):
                src, dl = data[d][r][c]
                j0, j1 = int(spans[d, r, t, 0]), int(spans[d, r, t, 1])
                for j in range(j0, j1):
                    ed = dl[j * 128:(j + 1) * 128]
                    es = src[j * 128:(j + 1) * 128]
                    in_tile = (ed >= 0) & ((ed >> 7) == t)
                    be = np.full(128, PAD_DL, np.int64)
                    bo = np.full(128, PAD_DL, np.int64)
                    par = (es & 1).astype(bool)
                    sel_e = in_tile & ~par
                    sel_o = in_tile & par
                    be[sel_e] = ed[sel_e] & 127
                    bo[sel_o] = ed[sel_o] & 127
                    dl_parts.append(np.stack([be, bo], axis=0).T)
    dstloc = np.ascontiguousarray(
        np.concatenate(dl_parts, axis=1).astype(BF16))
    return idx16, dstloc


# ======================================================================
# Launch 1: pair tables  y_{side,r}  [NPAIRPAD, 128] bf16
# ======================================================================

def build_prep_nc():
    nc = bass.Bass()
    feat_in = nc.dram_tensor("feat_slice", (2, NPAD, F), f32, kind="ExternalInput")
    cj_in = nc.dram_tensor("cj_slice", (2, 128, NT), f32, kind="ExternalInput")
    att_in = nc.dram_tensor("att", (R, NB), f32, kind="ExternalInput")
    basis_in = nc.dram_tensor("basis", (NB, F, F), f32, kind="ExternalInput")
    fcw_in = nc.dram_tensor("fc_w", (F, OUT), f32, kind="ExternalInput")
    y_out = nc.dram_tensor("y_slice", (2, R, NPAIR_CP, 128), bf16,
                           kind="ExternalOutput")

    ident_c = nc.inline_tensor(np.eye(128, dtype=np.float32), "ident_c")
    ones_c = nc.inline_tensor(np.ones((1, 128), dtype=np.float32), "ones_c")
    mult = mybir.AluOpType.mult
    add = mybir.AluOpType.add

    with tile.TileContext(nc) as tc:
        with (
            tc.tile_pool(name="cp", bufs=1) as cp,
            tc.tile_pool(name="sb", bufs=4) as sb,
            tc.tile_pool(name="xp", bufs=6) as xp,
            tc.tile_pool(name="ps", bufs=2, space="PSUM") as ps,
            tc.tile_pool(name="ps2", bufs=2, space="PSUM") as ps2,
        ):
            ident_t = cp.tile([128, 128], f32, tag="ident")
            nc.sync.dma_start(out=ident_t[:], in_=ident_c[:, :])
            ones_f32 = cp.tile([1, 128], f32, tag="ones32")
            nc.sync.dma_start(out=ones_f32[:], in_=ones_c[:, :])
            fcw_t = cp.tile([128, OUT], f32, tag="fcw")
            nc.sync.dma_start(out=fcw_t[:], in_=fcw_in[:, :])
            att_row = cp.tile([1, R * NB], f32, tag="attrow")
            nc.sync.dma_start(out=att_row[:],
                              in_=att_in[:, :].rearrange("r b -> () (r b)"))
            cj_sb = cp.tile([128, 2 * NT], f32, tag="cj")
            nc.sync.dma_start(
                out=cj_sb[:].rearrange("p (s t) -> p s t", s=2),
                in_=cj_in[:, :, :].rearrange("s p t -> p s t"))

            attb_ps = ps2.tile([128, R * NB], f32, tag="o2")
            nc.tensor.matmul(out=attb_ps[:], lhsT=ones_f32[:], rhs=att_row[:],
                             start=True, stop=True)
            att_b = cp.tile([128, R * NB], f32, tag="attb")
            nc.vector.tensor_copy(out=att_b[:], in_=attb_ps[:])

            bT = []
            for b in range(NB):
                bt_in = sb.tile([128, 128], f32, tag="bload")
                nc.sync.dma_start(out=bt_in[:], in_=basis_in[b, :, :])
                bt_ps = ps.tile([128, 128], f32, tag="tp")
                nc.tensor.transpose(out=bt_ps[:], in_=bt_in[:], identity=ident_t[:])
                bt_sb = cp.tile([128, 128], f32, tag=f"bT{b}")
                nc.vector.tensor_copy(out=bt_sb[:], in_=bt_ps[:])
                bT.append(bt_sb)

            wfc = cp.tile([128, R * OUT], bf16, tag="wfc")
            for r in range(R):
                wrt = sb.tile([128, 128], f32, tag="wrt")
                tmp = sb.tile([128, 128], f32, tag="wtmp")
                nc.vector.tensor_tensor(
                    out=tmp[:], in0=bT[1][:],
                    in1=att_b[:, 2 * r + 1:2 * r + 2].to_broadcast([128, 128]),
                    op=mult)
                nc.vector.tensor_tensor(
                    out=wrt[:], in0=bT[0][:],
                    in1=att_b[:, 2 * r:2 * r + 1].to_broadcast([128, 128]),
                    op=mult)
                nc.vector.tensor_tensor(out=wrt[:], in0=wrt[:], in1=tmp[:], op=add)
                wfc_ps = ps2.tile([128, OUT], f32, tag="o2")
                nc.tensor.matmul(out=wfc_ps[:], lhsT=wrt[:], rhs=fcw_t[:],
                                 start=True, stop=True)
                nc.scalar.copy(out=wfc[:, r * OUT:(r + 1) * OUT], in_=wfc_ps[:])

            for s in range(2):
                for t in range(NT):
                    rows = slice(t * 128, (t + 1) * 128)
                    ft = xp.tile([128, F], f32, tag="ft")
                    nc.sync.dma_start(out=ft[:], in_=feat_in[s, rows, :])
                    xt = xp.tile([128, F], f32, tag="xt")
                    nc.vector.tensor_tensor(
                        out=xt[:], in0=ft[:],
                        in1=cj_sb[:, s * NT + t:s * NT + t + 1].to_broadcast(
                            [128, F]),
                        op=mult)
                    xT_ps = ps.tile([128, 128], f32, tag="tp")
                    nc.tensor.transpose(out=xT_ps[:], in_=xt[:],
                                        identity=ident_t[:])
                    xT = xp.tile([128, 128], bf16, tag="xT")
                    nc.scalar.copy(out=xT[:], in_=xT_ps[:])
                    y_ps = ps.tile([128, R * OUT], f32, tag="y")
                    nc.tensor.matmul(out=y_ps[:], lhsT=xT[:], rhs=wfc[:],
                                     start=True, stop=True)
                    y_sb = xp.tile([128, R * OUT], bf16, tag="ysb")
                    nc.vector.tensor_copy(out=y_sb[:], in_=y_ps[:])
                    nc.sync.dma_start(
                        out=y_out[s, :, t * 64:(t + 1) * 64, :].rearrange(
                            "r q (h o) -> (q h) r o", h=2),
                        in_=y_sb[:].rearrange("p (r o) -> p r o", r=R))
    return nc


# ======================================================================
# Launch 2: main kernel (v4)
# ======================================================================

def build_main_nc(NCH, spans, icols, dcols, reps=1):
    nwin = [[int((NCH[d, r] + WCH - 1) // WCH) for r in range(R)]
            for d in range(2)]
    nc2t_all = [[2 * int(sum(spans[d, r, t, 1] - spans[d, r, t, 0]
                             for r in range(R)))
                 for t in range(NT)] for d in range(2)]
    nc2t_max = max(max(row) for row in nc2t_all)

    nc = bass.Bass(num_swdge_queues=NQ)
    ytab = [[nc.dram_tensor(f"y{s}{r}", (NPAIRPAD, 128), bf16,
                            kind="ExternalInput")
             for r in range(R)] for s in range(2)]
    idx_in = nc.dram_tensor("idx16", (128, icols), i16, kind="ExternalInput")
    dl_in = nc.dram_tensor("dstloc", (128, dcols), bf16, kind="ExternalInput")
    ci_in = nc.dram_tensor("ci_pad", (2, 128, NT), f32, kind="ExternalInput")
    fcb_in = nc.dram_tensor("fc_b", (OUT,), f32, kind="ExternalInput")
    out = nc.dram_tensor("out_part", (2, NPAD, R, OUT), f32,
                         kind="ExternalOutput")

    iota_np = np.broadcast_to(np.arange(128, dtype=np.float32), (128, 128))
    iota_c = nc.inline_tensor(np.ascontiguousarray(iota_np), "iota_c")
    ones_c = nc.inline_tensor(np.ones((1, 128), dtype=np.float32), "ones_c")

    eq = mybir.AluOpType.is_equal
    add = mybir.AluOpType.add
    NI = WCH * 128
    ICW = NI // 16

    with tile.TileContext(nc) as tc:
        with (
            tc.tile_pool(name="cp", bufs=1) as cp,
            tc.tile_pool(name="ixp", bufs=4) as ixp,
            tc.tile_pool(name="mp0", bufs=2) as mp0,
            tc.tile_pool(name="mp1", bufs=2) as mp1,
            tc.tile_pool(name="mp2", bufs=2) as mp2,
            tc.tile_pool(name="mp3", bufs=2) as mp3,
            tc.tile_pool(name="mp4", bufs=2) as mp4,
            tc.tile_pool(name="pp", bufs=2) as pp,
            tc.tile_pool(name="obp", bufs=4) as obp,
            tc.tile_pool(name="ps2", bufs=4, space="PSUM") as ps2,
        ):
            mps = [mp0, mp1, mp2, mp3, mp4]
            iota_f = cp.tile([128, 128], f32, tag="iotaf")
            nc.sync.dma_start(out=iota_f[:], in_=iota_c[:, :])
            iota_t = cp.tile([128, 128], bf16, tag="iota")
            nc.vector.tensor_copy(out=iota_t[:], in_=iota_f[:])
            iota2 = None
            if PSTYLE == "swap":
                iota2 = cp.tile([128, 128, nc2t_max], bf16, tag="iota2")
                nc.vector.tensor_copy(
                    out=iota2[:],
                    in_=iota_t[:, :, None].to_broadcast([128, 128, nc2t_max]))
            ones_f32 = cp.tile([1, 128], f32, tag="ones32")
            nc.sync.dma_start(out=ones_f32[:], in_=ones_c[:, :])
            fcb_row = cp.tile([1, OUT], f32, tag="fcbrow")
            nc.sync.dma_start(out=fcb_row[:], in_=fcb_in[None, :])
            ci_sb = cp.tile([128, 2 * NT], f32, tag="ci")
            nc.sync.dma_start(
                out=ci_sb[:].rearrange("p (s t) -> p s t", s=2),
                in_=ci_in[:, :, :].rearrange("s p t -> p s t"))
            dl_t = cp.tile([128, dcols], bf16, tag="dl")
            nc.sync.dma_start(out=dl_t[:], in_=dl_in[:, :])

            biasb_ps = ps2.tile([128, OUT], f32, tag="o2s")
            nc.tensor.matmul(out=biasb_ps[:], lhsT=ones_f32[:], rhs=fcb_row[:],
                             start=True, stop=True)
            bias5 = cp.tile([128, R * OUT], f32, tag="bias5")
            for r in range(R):
                nc.vector.tensor_copy(out=bias5[:, r * OUT:(r + 1) * OUT],
                                      in_=biasb_ps[:])

            ni_regs = {NI: nc.gpsimd.to_reg(NI)}
            for d in range(2):
                for r in range(R):
                    nch = int(NCH[d, r])
                    w = nwin[d][r] - 1
                    ni = (min(WCH, nch - w * WCH)) * 128
                    if ni not in ni_regs:
                        ni_regs[ni] = nc.gpsimd.to_reg(ni)

            import contextlib
            loop_cm = tc.For_i(0, reps) if reps > 1 else contextlib.nullcontext()
            with loop_cm:
                iblk = 0
                dcol = 0
                qn = 0
                for d in range(2):
                    mt = [[None] * nwin[d][r] for r in range(R)]
                    wmax = max(nwin[d][r] for r in range(R))
                    boff = [iblk + sum(nwin[d][rr] for rr in range(r))
                            for r in range(R)]
                    for w in range(wmax):
                        for r in range(R):
                            if w >= nwin[d][r]:
                                continue
                            nch = int(NCH[d, r])
                            wch = (min(WCH, nch - w * WCH)
                                   if w == nwin[d][r] - 1 else WCH)
                            ni = wch * 128
                            ix = ixp.tile([128, ICW], i16, tag="ix")
                            nc.sync.dma_start(
                                out=ix[:, :ni // 16],
                                in_=idx_in[:, (boff[r] + w) * ICW:
                                           (boff[r] + w) * ICW + ni // 16])
                            m = mps[r].tile([128, WCH * 128], bf16, tag="m")
                            nc.gpsimd.dma_gather(
                                m[:, :wch * 128].rearrange(
                                    "p (c f) -> p c f", c=wch),
                                ytab[d][r][:, :],
                                ix[:, :ni // 16],
                                ni, ni_regs[ni], 128,
                                single_packet=False,
                                queue_num=qn % NQ)
                            qn += 1
                            mt[r][w] = m
                    iblk += sum(nwin[d][r] for r in range(R))

                    p_tiles = {}
                    dcols_t = [dcol]
                    for t in range(NT):
                        dcols_t.append(dcols_t[-1] + nc2t_all[d][t])

                    def build_p(t):
                        nc2t = nc2t_all[d][t]
                        if PSTYLE == "swap":
                            p_t = pp.tile([128, 128, nc2t_max], bf16, tag="p")
                            nc.vector.tensor_tensor(
                                out=p_t[:, :, :nc2t],
                                in0=dl_t[:, dcols_t[t]:dcols_t[t] + nc2t]
                                [:, None, :].to_broadcast([128, 128, nc2t]),
                                in1=iota2[:, :, :nc2t],
                                op=eq)
                        else:
                            p_t = pp.tile([128, nc2t_max, 128], bf16, tag="p")
                            nc.vector.tensor_tensor(
                                out=p_t[:, :nc2t, :],
                                in0=dl_t[:, dcols_t[t]:dcols_t[t] + nc2t]
                                [:, :, None].to_broadcast([128, nc2t, 128]),
                                in1=iota_t[:, None, :].to_broadcast(
                                    [128, nc2t, 128]),
                                op=eq)
                        p_tiles[t] = p_t

                    def p_col(p_t, col):
                        if PSTYLE == "swap":
                            return p_t[:, :, col]
                        return p_t[:, col, :]

                    build_p(0)
                    for t in range(NT):
                        if t + 1 < NT:
                            build_p(t + 1)
                        p_t = p_tiles.pop(t)
                        o2 = ps2.tile([128, R * OUT], f32, tag="o2")
                        pc = 0
                        for r in range(R):
                            j0 = int(spans[d, r, t, 0])
                            j1 = int(spans[d, r, t, 1])
                            k = j1 - j0
                            for jj in range(k):
                                j = j0 + jj
                                w, q = divmod(j, WCH)
                                for par in range(2):
                                    nc.tensor.matmul(
                                        out=o2[:, r * OUT:(r + 1) * OUT],
                                        lhsT=p_col(p_t, 2 * (pc + jj) + par),
                                        rhs=mt[r][w][:, q * 128 + par * 64:
                                                     q * 128 + par * 64 + 64],
                                        start=(jj == 0 and par == 0),
                                        stop=(jj == k - 1 and par == 1))
                            pc += k
                        dcol += 2 * pc
                        ob = obp.tile([128, R * OUT], f32, tag="ob")
                        nc.scalar.activation(
                            out=ob[:], in_=o2[:],
                            func=mybir.ActivationFunctionType.Copy,
                            scale=ci_sb[:, d * NT + t:d * NT + t + 1])
                        nc.vector.tensor_tensor(out=ob[:], in0=ob[:],
                                                in1=bias5[:], op=add)
                        nc.scalar.dma_start(
                            out=out[d, t * 128:(t + 1) * 128, :, :].rearrange(
                                "p r o -> p (r o)"),
                            in_=ob[:])
    _finalize_libraries(nc)
    return nc




# ======================================================================
# Fused single-launch kernel: replicated table build + gather/aggregate
# ======================================================================

NTF = NPAIRPAD * 2 // 128      # full node tiles per side (391)


def build_fused_nc(NCH, spans, icols, dcols, reps=1):
    nwin = [[int((NCH[d, r] + WCH - 1) // WCH) for r in range(R)]
            for d in range(2)]
    nc2t_all = [[2 * int(sum(spans[d, r, t, 1] - spans[d, r, t, 0]
                             for r in range(R)))
                 for t in range(NT)] for d in range(2)]
    nc2t_max = max(max(row) for row in nc2t_all)

    nc = bass.Bass(num_swdge_queues=NQ)
    feat_in = nc.dram_tensor("feat_full", (2, NTF * 128, F), f32,
                             kind="ExternalInput")
    cj_in = nc.dram_tensor("cj_full", (2, 128, NTF), f32,
                           kind="ExternalInput")
    att_in = nc.dram_tensor("att", (R, NB), f32, kind="ExternalInput")
    basis_in = nc.dram_tensor("basis", (NB, F, F), f32, kind="ExternalInput")
    fcw_in = nc.dram_tensor("fc_w", (F, OUT), f32, kind="ExternalInput")
    yts = [nc.dram_tensor(f"ytall{side}", (R, NPAIRPAD, 128), bf16,
                          kind="Internal") for side in range(2)]
    idx_in = nc.dram_tensor("idx16", (128, icols), i16, kind="ExternalInput")
    dl_in = nc.dram_tensor("dstloc", (128, dcols), bf16, kind="ExternalInput")
    ci_in = nc.dram_tensor("ci_pad", (2, 128, NT), f32, kind="ExternalInput")
    fcb_in = nc.dram_tensor("fc_b", (OUT,), f32, kind="ExternalInput")
    out = nc.dram_tensor("out_part", (2, NPAD, R, OUT), f32,
                         kind="ExternalOutput")

    iota_np = np.broadcast_to(np.arange(128, dtype=np.float32), (128, 128))
    iota_c = nc.inline_tensor(np.ascontiguousarray(iota_np), "iota_c")
    ident_c = nc.inline_tensor(np.eye(128, dtype=np.float32), "ident_c")
    ones_c = nc.inline_tensor(np.ones((1, 128), dtype=np.float32), "ones_c")

    eq = mybir.AluOpType.is_equal
    add = mybir.AluOpType.add
    mult = mybir.AluOpType.mult
    NI = WCH * 128
    ICW = NI // 16

    with tile.TileContext(nc) as tc:
        with (
            tc.tile_pool(name="cp", bufs=1) as cp,
            tc.tile_pool(name="xp", bufs=4) as xp,
            tc.tile_pool(name="sb", bufs=2) as sb,
            tc.tile_pool(name="ixp", bufs=4) as ixp,
            tc.tile_pool(name="mp0", bufs=2) as mp0,
            tc.tile_pool(name="mp1", bufs=2) as mp1,
            tc.tile_pool(name="mp2", bufs=2) as mp2,
            tc.tile_pool(name="mp3", bufs=2) as mp3,
            tc.tile_pool(name="mp4", bufs=2) as mp4,
            tc.tile_pool(name="pp", bufs=2) as pp,
            tc.tile_pool(name="obp", bufs=4) as obp,
            tc.tile_pool(name="psA", bufs=2, space="PSUM") as psA,
            tc.tile_pool(name="ps2", bufs=2, space="PSUM") as ps2,
        ):
            mps = [mp0, mp1, mp2, mp3, mp4]
            # ---------- shared consts ----------
            iota_f = cp.tile([128, 128], f32, tag="iotaf")
            nc.sync.dma_start(out=iota_f[:], in_=iota_c[:, :])
            iota_t = cp.tile([128, 128], bf16, tag="iota")
            nc.vector.tensor_copy(out=iota_t[:], in_=iota_f[:])
            iota2 = cp.tile([128, 128, nc2t_max], bf16, tag="iota2")
            nc.vector.tensor_copy(
                out=iota2[:],
                in_=iota_t[:, :, None].to_broadcast([128, 128, nc2t_max]))
            ident_t = cp.tile([128, 128], f32, tag="ident")
            nc.sync.dma_start(out=ident_t[:], in_=ident_c[:, :])
            ones_f32 = cp.tile([1, 128], f32, tag="ones32")
            nc.sync.dma_start(out=ones_f32[:], in_=ones_c[:, :])
            fcw_t = cp.tile([128, OUT], f32, tag="fcw")
            nc.sync.dma_start(out=fcw_t[:], in_=fcw_in[:, :])
            att_row = cp.tile([1, R * NB], f32, tag="attrow")
            nc.sync.dma_start(out=att_row[:],
                              in_=att_in[:, :].rearrange("r b -> () (r b)"))
            cj_sb = cp.tile([128, 2 * NTF], f32, tag="cjf")
            nc.sync.dma_start(
                out=cj_sb[:].rearrange("p (s t) -> p s t", s=2),
                in_=cj_in[:, :, :].rearrange("s p t -> p s t"))
            fcb_row = cp.tile([1, OUT], f32, tag="fcbrow")
            nc.sync.dma_start(out=fcb_row[:], in_=fcb_in[None, :])
            ci_sb = cp.tile([128, 2 * NT], f32, tag="ci")
            nc.sync.dma_start(
                out=ci_sb[:].rearrange("p (s t) -> p s t", s=2),
                in_=ci_in[:, :, :].rearrange("s p t -> p s t"))
            dl_t = cp.tile([128, dcols], bf16, tag="dl")
            nc.sync.dma_start(out=dl_t[:], in_=dl_in[:, :])

            # Wfc_r = (sum_b att[r,b] basis[b]^T)^T-style precompute (as in
            # the standalone prep kernel)
            attb_ps = ps2.tile([128, R * NB], f32, tag="o2s")
            nc.tensor.matmul(out=attb_ps[:], lhsT=ones_f32[:], rhs=att_row[:],
                             start=True, stop=True)
            att_b = cp.tile([128, R * NB], f32, tag="attb")
            nc.vector.tensor_copy(out=att_b[:], in_=attb_ps[:])
            bT = []
            for b in range(NB):
                bt_in = sb.tile([128, 128], f32, tag="bload")
                nc.sync.dma_start(out=bt_in[:], in_=basis_in[b, :, :])
                bt_ps = psA.tile([128, 128], f32, tag="tp")
                nc.tensor.transpose(out=bt_ps[:], in_=bt_in[:],
                                    identity=ident_t[:])
                bt_sb = cp.tile([128, 128], f32, tag=f"bT{b}")
                nc.vector.tensor_copy(out=bt_sb[:], in_=bt_ps[:])
                bT.append(bt_sb)
            wfc = cp.tile([128, R * OUT], bf16, tag="wfc")
            for r in range(R):
                wrt = sb.tile([128, 128], f32, tag="wrt")
                tmp = sb.tile([128, 128], f32, tag="wtmp")
                nc.vector.tensor_tensor(
                    out=tmp[:], in0=bT[1][:],
                    in1=att_b[:, 2 * r + 1:2 * r + 2].to_broadcast([128, 128]),
                    op=mult)
                nc.vector.tensor_tensor(
                    out=wrt[:], in0=bT[0][:],
                    in1=att_b[:, 2 * r:2 * r + 1].to_broadcast([128, 128]),
                    op=mult)
                nc.vector.tensor_tensor(out=wrt[:], in0=wrt[:], in1=tmp[:],
                                        op=add)
                wfc_ps = ps2.tile([128, OUT], f32, tag="o2s")
                nc.tensor.matmul(out=wfc_ps[:], lhsT=wrt[:], rhs=fcw_t[:],
                                 start=True, stop=True)
                nc.scalar.copy(out=wfc[:, r * OUT:(r + 1) * OUT], in_=wfc_ps[:])

            biasb_ps = ps2.tile([128, OUT], f32, tag="o2s")
            nc.tensor.matmul(out=biasb_ps[:], lhsT=ones_f32[:], rhs=fcb_row[:],
                             start=True, stop=True)
            bias5 = cp.tile([128, R * OUT], f32, tag="bias5")
            for r in range(R):
                nc.vector.tensor_copy(out=bias5[:, r * OUT:(r + 1) * OUT],
                                      in_=biasb_ps[:])

            ni_regs = {NI: nc.gpsimd.to_reg(NI)}
            for d in range(2):
                for r in range(R):
                    nch = int(NCH[d, r])
                    w = nwin[d][r] - 1
                    ni = (min(WCH, nch - w * WCH)) * 128
                    if ni not in ni_regs:
                        ni_regs[ni] = nc.gpsimd.to_reg(ni)

            def prep_side(side):
                # full-table transform for one side (replicated per core)
                for t in range(NTF):
                    rows = slice(t * 128, (t + 1) * 128)
                    ft = xp.tile([128, F], f32, tag="ft")
                    nc.sync.dma_start(out=ft[:], in_=feat_in[side, rows, :])
                    xt = xp.tile([128, F], f32, tag="xt")
                    nc.vector.tensor_tensor(
                        out=xt[:], in0=ft[:],
                        in1=cj_sb[:, side * NTF + t:side * NTF + t + 1]
                        .to_broadcast([128, F]),
                        op=mult)
                    xT_ps = psA.tile([128, 128], f32, tag="tp")
                    nc.tensor.transpose(out=xT_ps[:], in_=xt[:],
                                        identity=ident_t[:])
                    xT = xp.tile([128, 128], bf16, tag="xT")
                    nc.scalar.copy(out=xT[:], in_=xT_ps[:])
                    y_ps = psA.tile([128, R * OUT], f32, tag="y")
                    nc.tensor.matmul(out=y_ps[:], lhsT=xT[:], rhs=wfc[:],
                                     start=True, stop=True)
                    y_sb = xp.tile([128, R * OUT], bf16, tag="ysb")
                    nc.vector.tensor_copy(out=y_sb[:], in_=y_ps[:])
                    nc.sync.dma_start(
                        out=yts[side][:, t * 64:(t + 1) * 64, :].rearrange(
                            "r q (h o) -> (q h) r o", h=2),
                        in_=y_sb[:].rearrange("p (r o) -> p r o", r=R))

            import contextlib
            loop_cm = tc.For_i(0, reps) if reps > 1 else contextlib.nullcontext()
            with loop_cm:
                iblk = 0
                dcol = 0
                qn = 0
                prep_side(0)       # d0 sources side 0
                prep_side(1)       # overlaps d0 gathers (PE order precedes
                                   # d0 matmuls; gathers are Pool-side)
                for d in range(2):
                    mt = [[None] * nwin[d][r] for r in range(R)]
                    wmax = max(nwin[d][r] for r in range(R))
                    boff = [iblk + sum(nwin[d][rr] for rr in range(r))
                            for r in range(R)]
                    for w in range(wmax):
                        for r in range(R):
                            if w >= nwin[d][r]:
                                continue
                            nch = int(NCH[d, r])
                            wch = (min(WCH, nch - w * WCH)
                                   if w == nwin[d][r] - 1 else WCH)
                            ni = wch * 128
                            ix = ixp.tile([128, ICW], i16, tag="ix")
                            nc.sync.dma_start(
                                out=ix[:, :ni // 16],
                                in_=idx_in[:, (boff[r] + w) * ICW:
                                           (boff[r] + w) * ICW + ni // 16])
                            m = mps[r].tile([128, WCH * 128], bf16, tag="m")
                            nc.gpsimd.dma_gather(
                                m[:, :wch * 128].rearrange(
                                    "p (c f) -> p c f", c=wch),
                                yts[d][r],
                                ix[:, :ni // 16],
                                ni, ni_regs[ni], 128,
                                single_packet=False,
                                queue_num=qn % NQ)
                            qn += 1
                            mt[r][w] = m
                    iblk += sum(nwin[d][r] for r in range(R))

                    p_tiles = {}
                    dcols_t = [dcol]
                    for t in range(NT):
                        dcols_t.append(dcols_t[-1] + nc2t_all[d][t])

                    def build_p(t):
                        nc2t = nc2t_all[d][t]
                        p_t = pp.tile([128, 128, nc2t_max], bf16, tag="p")
                        nc.vector.tensor_tensor(
                            out=p_t[:, :, :nc2t],
                            in0=dl_t[:, dcols_t[t]:dcols_t[t] + nc2t]
                            [:, None, :].to_broadcast([128, 128, nc2t]),
                            in1=iota2[:, :, :nc2t],
                            op=eq)
                        p_tiles[t] = p_t

                    build_p(0)
                    for t in range(NT):
                        if t + 1 < NT:
                            build_p(t + 1)
                        p_t = p_tiles.pop(t)
                        o2 = ps2.tile([128, R * OUT], f32, tag="o2")
                        pc = 0
                        for r in range(R):
                            j0 = int(spans[d, r, t, 0])
                            j1 = int(spans[d, r, t, 1])
                            k = j1 - j0
                            for jj in range(k):
                                j = j0 + jj
                                w, q = divmod(j, WCH)
                                for par in range(2):
                                    nc.tensor.matmul(
                                        out=o2[:, r * OUT:(r + 1) * OUT],
                                        lhsT=p_t[:, :, 2 * (pc + jj) + par],
                                        rhs=mt[r][w][:, q * 128 + par * 64:
                                                     q * 128 + par * 64 + 64],
                                        start=(jj == 0 and par == 0),
                                        stop=(jj == k - 1 and par == 1))
                            pc += k
                        dcol += 2 * pc
                        ob = obp.tile([128, R * OUT], f32, tag="ob")
                        nc.scalar.activation(
                            out=ob[:], in_=o2[:],
                            func=mybir.ActivationFunctionType.Copy,
                            scale=ci_sb[:, d * NT + t:d * NT + t + 1])
                        nc.vector.tensor_tensor(out=ob[:], in0=ob[:],
                                                in1=bias5[:], op=add)
                        nc.scalar.dma_start(
                            out=out[d, t * 128:(t + 1) * 128, :, :].rearrange(
                                "p r o -> p (r o)"),
                            in_=ob[:])
    _finalize_libraries(nc)
    return nc


# ======================================================================
# kernel entry
# ======================================================================

_cache: dict = {}

FUSED = False      # two-launch path measured faster (fused: WAR-serialized prep)


def _kernel_fused(drug_feat, dis_feat, cj_drug, ci_drug, cj_dis, ci_dis,
                  att, basis, fc_w, fc_b, edge_drug, edge_dis):
    _tlog("start (fused)")
    NCH, spans, data = prep_edges(edge_drug, edge_dis)
    arrays = [build_core_arrays(NCH, spans, data, c) for c in range(NCORES)]
    icols = arrays[0][0].shape[1]
    dcols = arrays[0][1].shape[1]
    _tlog(f"host edge prep done (chunks={int(NCH.sum())})")

    key = ("fused", NCH.tobytes(), spans.tobytes(), icols, dcols)
    if key not in _cache:
        _cache[key] = build_fused_nc(NCH, spans, icols, dcols)
        _tlog("fused program built")
    nc = _cache[key]

    feat_full = np.zeros((2, NTF * 128, F), np.float32)
    feat_full[0, :N] = drug_feat
    feat_full[1, :N] = dis_feat
    cj_full = np.zeros((2, NTF * 128), np.float32)
    cj_full[0, :N] = cj_drug
    cj_full[1, :N] = cj_dis
    cj_full = np.ascontiguousarray(
        cj_full.reshape(2, NTF, 128).transpose(0, 2, 1))

    in_maps = []
    for c in range(NCORES):
        rows = slice(c * NPC, (c + 1) * NPC)
        ci_pad = np.zeros((2, NPAD), np.float32)
        ci_pad[0, :NPC] = ci_dis[rows]
        ci_pad[1, :NPC] = ci_drug[rows]
        ci_pad = np.ascontiguousarray(
            ci_pad.reshape(2, NT, 128).transpose(0, 2, 1))
        in_maps.append({
            "feat_full": feat_full, "cj_full": cj_full, "att": att,
            "basis": basis, "fc_w": fc_w, "fc_b": fc_b,
            "idx16": arrays[c][0], "dstloc": arrays[c][1],
            "ci_pad": ci_pad})
    _tlog("inputs built")
    res = run_bass_kernel_spmd(nc, in_maps, core_ids=list(range(NCORES)))
    _tlog("launch done")
    out_dis = np.concatenate(
        [r["out_part"][0, :NPC] for r in res.results], axis=0)
    out_drug = np.concatenate(
        [r["out_part"][1, :NPC] for r in res.results], axis=0)
    return out_drug.astype(np.float32), out_dis.astype(np.float32)


def kernel(drug_feat, dis_feat, cj_drug, ci_drug, cj_dis, ci_dis,
           att, basis, fc_w, fc_b, edge_drug, edge_dis):
    drug_feat = np.asarray(drug_feat, np.float32)
    dis_feat = np.asarray(dis_feat, np.float32)
    cj_drug = np.asarray(cj_drug, np.float32)
    ci_drug = np.asarray(ci_drug, np.float32)
    cj_dis = np.asarray(cj_dis, np.float32)
    ci_dis = np.asarray(ci_dis, np.float32)
    att = np.asarray(att, np.float32)
    basis = np.asarray(basis, np.float32)
    fc_w = np.asarray(fc_w, np.float32)
    fc_b = np.asarray(fc_b, np.float32)
    edge_drug = np.asarray(edge_drug, np.int32)
    edge_dis = np.asarray(edge_dis, np.int32)

    if FUSED:
        return _kernel_fused(drug_feat, dis_feat, cj_drug, ci_drug, cj_dis,
                             ci_dis, att, basis, fc_w, fc_b,
                             edge_drug, edge_dis)
    _tlog("start")
    NCH, spans, data = prep_edges(edge_drug, edge_dis)
    arrays = [build_core_arrays(NCH, spans, data, c) for c in range(NCORES)]
    icols = arrays[0][0].shape[1]
    dcols = arrays[0][1].shape[1]
    _tlog(f"host edge prep done (chunks={int(NCH.sum())}, icols={icols})")

    # ---- launch 1 ----
    if "prep" not in _cache:
        _cache["prep"] = build_prep_nc()
    nc1 = _cache["prep"]
    in_maps1 = []
    for c in range(NCORES):
        rows = slice(c * NPC, (c + 1) * NPC)
        feat_slice = np.zeros((2, NPAD, F), np.float32)
        feat_slice[0, :NPC] = drug_feat[rows]
        feat_slice[1, :NPC] = dis_feat[rows]
        cj_slice = np.zeros((2, NPAD), np.float32)
        cj_slice[0, :NPC] = cj_drug[rows]
        cj_slice[1, :NPC] = cj_dis[rows]
        cj_slice = np.ascontiguousarray(
            cj_slice.reshape(2, NT, 128).transpose(0, 2, 1))
        in_maps1.append({"feat_slice": feat_slice, "cj_slice": cj_slice,
                         "att": att, "basis": basis, "fc_w": fc_w})
    _tlog("launch1 inputs built")
    res1 = run_bass_kernel_spmd(nc1, in_maps1, core_ids=list(range(NCORES)))
    _tlog("launch1 done")

    ytabs = np.zeros((2, R, NPAIRPAD, 128), BF16)
    for c in range(NCORES):
        ytabs[:, :, c * NPAIR_C:(c + 1) * NPAIR_C, :] = \
            res1.results[c]["y_slice"][:, :, :NPAIR_C]
    _tlog("tables assembled")

    # ---- launch 2 ----
    key = ("main", NCH.tobytes(), spans.tobytes(), icols, dcols)
    if key not in _cache:
        _cache[key] = build_main_nc(NCH, spans, icols, dcols)
        _tlog("launch2 program built")
    nc2 = _cache[key]

    in_maps2 = []
    for c in range(NCORES):
        rows = slice(c * NPC, (c + 1) * NPC)
        ci_pad = np.zeros((2, NPAD), np.float32)
        ci_pad[0, :NPC] = ci_dis[rows]    # dir 0 dest = dis
        ci_pad[1, :NPC] = ci_drug[rows]   # dir 1 dest = drug
        ci_pad = np.ascontiguousarray(
            ci_pad.reshape(2, NT, 128).transpose(0, 2, 1))
        im = {"idx16": arrays[c][0], "dstloc": arrays[c][1],
              "ci_pad": ci_pad, "fc_b": fc_b}
        for s in range(2):
            for r in range(R):
                im[f"y{s}{r}"] = ytabs[s, r]
        in_maps2.append(im)
    _tlog("launch2 inputs built")
    res2 = run_bass_kernel_spmd(nc2, in_maps2, core_ids=list(range(NCORES)))
    _tlog("launch2 done")

    out_dis = np.concatenate(
        [r["out_part"][0, :NPC] for r in res2.results], axis=0)
    out_drug = np.concatenate(
        [r["out_part"][1, :NPC] for r in res2.results], axis=0)
    _tlog("assembled")
    return out_drug.astype(np.float32), out_dis.astype(np.float32)
